# revision 16
# baseline (speedup 1.0000x reference)
"""Trainium2 Bass kernel for nn_AttnBFAN (batched attention w/ focal re-norm).

Data-parallel over the batch dim: 128 batches sharded 16-per-core across 8
NeuronCores. Per batch (Q=128, C=1024, D=1024):
    attn = leaky_relu(context @ query^T, 0.1)          (C, Q)
    attn = attn / (||attn||_2 over q)                  l2norm per (b, c)
    p    = softmax(20 * attn^T, axis=c)                (Q, C)
    t    = (p > mean_c p) * p ; re_attn = t / sum_c t
    wcontext = re_attn @ context                       (Q, D)
returns (query, wcontext, re_attn).

v4: bf16 matmul path + aggressive software pipelining.
 - Host pre-casts context/query to bf16 (halves HBM traffic; rel err vs
   fp32 reference ~6e-3) and pre-transposes query to [d, q] chunks.
 - All PE transposes and bmms run in bf16 at 1.0 cycle/row. The l2-norm
   / softmax / focal chain stays f32 (per-c-column norm errors don't
   cancel in the softmax).
 - bmm2 multiplies the unnormalized focal weights t (bf16) and folds
   the 1/sum_c(t) renorm into the PSUM eviction (per-partition scale).
 - PE stream per batch: [ctx^T chunks for b+1] | ones(b) | bmm1(b+1) |
   t^T(b) | bmm2(b) — bmm1 of the next batch fills the softmax-chain
   gap. The l2/softmax PSUM (S) lives in the bmm2 banks so bmm1(b+1)
   can take a0/a1 right after the Prelu eviction.
 - Loads/stores spread over the three dynamic DMA queues (gpsimd SWDGE,
   SP HWDGE, ACT HWDGE).
"""

import os
import numpy as np
import ml_dtypes

import concourse.bacc as bacc
import concourse.mybir as mybir
import concourse.tile as tile
from concourse.bass_utils import run_bass_kernel_spmd
from concourse.masks import make_identity
from concourse.hw_specs import get_activation_tables

F32 = mybir.dt.float32
F32R = mybir.dt.float32r
BF16 = mybir.dt.bfloat16
AX = mybir.AxisListType
ALU = mybir.AluOpType
ACTF = mybir.ActivationFunctionType

NCORES = 8
NB = 128          # total batches
BPC = NB // NCORES  # batches per core
Q = 128
C = 1024
D = 1024
SMOOTH = 20.0

_CACHE = {}


def _build():
    nc = bacc.Bacc("TRN2", target_bir_lowering=False, debug=False,
                   num_devices=NCORES, name="attn_bfan")
    # query pre-transposed+tiled on host: [b, p(=d%128), jd, q] bf16
    q_in = nc.dram_tensor("query", [BPC, 128, 8, Q], BF16, kind="ExternalInput")
    # context pre-tiled on host: [b, p(=c%128), jc, d] bf16 so each
    # partition's DMA line is 16 KB contiguous (descriptor-overhead bound
    # otherwise: 2 KB bf16 rows halve effective DMA rate)
    c_in = nc.dram_tensor("context", [BPC, 128, 8, D], BF16, kind="ExternalInput")
    re_out = nc.dram_tensor("re_attn", [BPC, Q, C], F32, kind="ExternalOutput")
    wc_out = nc.dram_tensor("wcontext", [BPC, Q, D], F32, kind="ExternalOutput")

    with tile.TileContext(nc) as tc:
        with (
            tc.tile_pool(name="singles", bufs=1) as singles,
            tc.tile_pool(name="ctxp", bufs=3) as ctxp,
            tc.tile_pool(name="ctxtp", bufs=2) as ctxtp,
            tc.tile_pool(name="qp", bufs=2) as qp,
            tc.tile_pool(name="work", bufs=2) as work,
            tc.tile_pool(name="w1", bufs=1) as w1,
            tc.tile_pool(name="tpool", bufs=2) as tpool,
            tc.tile_pool(name="stat", bufs=2) as stat,
            tc.tile_pool(name="ps_a", bufs=1, space="PSUM") as ps_a,
            tc.tile_pool(name="ps_w", bufs=1, space="PSUM") as ps_w,
            tc.tile_pool(name="ps_tp", bufs=2, space="PSUM") as ps_tp,
        ):
            tab_names = list(get_activation_tables("gen3").keys())
            nc.scalar.add_instruction(mybir.InstLoadActFuncSet(
                name=nc.get_next_instruction_name(),
                act_func_set_id=tab_names.index("natural_log_exp_and_others"),
                ins=[], outs=[]))
            ident = singles.tile([128, 128], F32, tag="ident")
            make_identity(nc, ident[:])
            identb = singles.tile([128, 128], BF16, tag="identb")
            nc.vector.tensor_copy(identb[:], ident[:])
            ones_f = singles.tile([128, 128], F32, tag="ones_f")
            nc.vector.memset(ones_f[:], 1.0)
            ones_r = singles.tile([128, 128], F32R, tag="ones_r")
            nc.vector.tensor_copy(ones_r[:], ones_f[:])
            ln20 = singles.tile([128, 1], F32, tag="ln20")
            nc.vector.memset(ln20[:], float(np.log(SMOOTH)))
            invC = singles.tile([128, 1], F32, tag="invC")
            nc.vector.memset(invC[:], 1.0 / C)

            ctx_t = [None] * (BPC + 1)   # plain ctx bf16 [128, 8jc, 1024d]
            ctxT_t = [None] * (BPC + 1)  # ctx^T bf16 [128, 8jd, 1024c]
            qT_t = [None] * (BPC + 1)    # q^T bf16 [128, 8jd, 128q]

            def load_batch(b):
                ctx = ctxp.tile([128, 8, D], BF16, tag="ctx", name="ctx")
                nc.gpsimd.dma_start(out=ctx[:], in_=c_in[b])
                ctx_t[b] = ctx
                qT = qp.tile([128, 8, Q], BF16, tag="qT", name="qT")
                nc.gpsimd.dma_start(out=qT[:], in_=q_in[b])
                qT_t[b] = qT

            def transpose_pair(b, jd, copy_eng):
                # PE-transpose ctx chunks jd, jd+1 into one 2-bank PSUM tile,
                # evict with a single 2048-elem bf16 copy on copy_eng.
                ctx = ctx_t[b]
                if ctxT_t[b] is None:
                    ctxT_t[b] = ctxtp.tile([128, 8, C], BF16, tag="ctxT",
                                           name="ctxT")
                ctxT = ctxT_t[b]
                tp = ps_tp.tile([128, 16, 128], BF16, tag="tp", name="tp")
                for k in range(2):
                    for jc in range(8):
                        nc.tensor.transpose(
                            tp[:, k * 8 + jc, :],
                            ctx[:, jc, (jd + k) * 128:(jd + k + 1) * 128],
                            identb[:])
                src = tp[:].rearrange("p a b -> p (a b)")
                dst = ctxT[:, jd:jd + 2, :].rearrange("p a b -> p (a b)")
                if copy_eng == "act":
                    nc.scalar.copy(dst, src)
                else:
                    nc.vector.tensor_copy(dst, src)

            def bmm1(b):
                # attn^T (q, c) accumulated over 8 d-chunks -> a0/a1
                a0 = ps_a.tile([128, 512], F32, tag="a0", name="a0")
                a1 = ps_a.tile([128, 512], F32, tag="a1", name="a1")
                qT = qT_t[b]
                ctxT = ctxT_t[b]
                for jd in range(8):
                    st, sp = jd == 0, jd == 7
                    nc.tensor.matmul(a0[:], qT[:, jd, :], ctxT[:, jd, 0:512],
                                     start=st, stop=sp)
                    nc.tensor.matmul(a1[:], qT[:, jd, :], ctxT[:, jd, 512:1024],
                                     start=st, stop=sp)
                return a0, a1

            # ---- prologue: batch 0 fully staged, batch 1 loading ----
            load_batch(0)
            load_batch(1)
            for jd in (0, 2, 4, 6):
                transpose_pair(0, jd, "vec" if jd != 0 else "act")
            a_cur = bmm1(0)

            for b in range(BPC):
                a0, a1 = a_cur
                if b + 2 < BPC:
                    load_batch(b + 2)

                # ---- leaky relu eviction (frees a0/a1 for bmm1(b+1)) ----
                attn = work.tile([128, C], F32, tag="attn")
                nc.scalar.activation(attn[:, 0:512], a0[:], ACTF.Prelu,
                                     bias=0.0, scale=1.0, alpha=0.1)
                nc.scalar.activation(attn[:, 512:1024], a1[:], ACTF.Prelu,
                                     bias=0.0, scale=1.0, alpha=0.1)
                # squares on DVE (f32r for the ones-matmul)
                sq = w1.tile([128, C], F32R, tag="w1a")
                nc.vector.tensor_mul(sq[:, 0:512], attn[:, 0:512], attn[:, 0:512])
                nc.vector.tensor_mul(sq[:, 512:1024], attn[:, 512:1024],
                                     attn[:, 512:1024])

                # next batch ctx^T chunks 0-3 (ACT copies the first pair in
                # its pre-Ln gap, DVE the second)
                if b + 1 < BPC:
                    transpose_pair(b + 1, 0, "act")
                    transpose_pair(b + 1, 2, "vec")

                # ---- l2 norm: ones-matmul into the bmm2 banks ----
                w0 = ps_w.tile([128, 512], F32, tag="w0", name="w0")
                w2 = ps_w.tile([128, 512], F32, tag="w2", name="w2")
                nc.tensor.matmul(w0[:], ones_r[:], sq[:, 0:512], start=True, stop=True)
                nc.tensor.matmul(w2[:], ones_r[:], sq[:, 512:1024], start=True, stop=True)

                if b + 1 < BPC:
                    transpose_pair(b + 1, 4, "vec")
                    transpose_pair(b + 1, 6, "vec")

                # 20/sqrt(S) = exp(-0.5*ln(S) + ln 20), half-split so ACT/DVE
                # ping-pong: Ln0 -> rn0 -> u0(DVE) -> pu0 while h1 follows
                lnS = w1.tile([128, C], F32, tag="w1b")
                nc.scalar.activation(lnS[:, 0:512], w0[:], ACTF.Ln)
                nc.scalar.activation(lnS[:, 512:1024], w2[:], ACTF.Ln)
                rn20 = w1.tile([128, C], F32, tag="w1c")
                u = w1.tile([128, C], F32, tag="w1a")
                pu = work.tile([128, C], F32, tag="pu")
                rs0 = stat.tile([128, 1], F32, tag="rs0")
                rs1 = stat.tile([128, 1], F32, tag="rs1")
                h0, h1 = slice(0, 512), slice(512, 1024)
                nc.scalar.activation(rn20[:, h0], lnS[:, h0], ACTF.Exp,
                                     bias=ln20[:], scale=-0.5)
                nc.scalar.activation(rn20[:, h1], lnS[:, h1], ACTF.Exp,
                                     bias=ln20[:], scale=-0.5)
                nc.vector.tensor_mul(u[:, h0], attn[:, h0], rn20[:, h0])
                nc.vector.tensor_mul(u[:, h1], attn[:, h1], rn20[:, h1])
                # pu = exp(20u), rs = sum_c pu per half
                nc.scalar.activation(pu[:, h0], u[:, h0], ACTF.Exp,
                                     bias=0.0, scale=1.0, accum_out=rs0[:])
                nc.scalar.activation(pu[:, h1], u[:, h1], ACTF.Exp,
                                     bias=0.0, scale=1.0, accum_out=rs1[:])
                # thr = (rs0 + rs1) / C in one DVE op
                thr = stat.tile([128, 1], F32, tag="thr")
                nc.vector.scalar_tensor_tensor(
                    out=thr[:], in0=rs0[:], scalar=rs1[:], in1=invC[:],
                    op0=ALU.add, op1=ALU.mult)

                # ---- focal: t = (pu > thr) * pu (bf16), ts = sum_c t ----
                t = tpool.tile([128, C], BF16, tag="t")
                ts = stat.tile([128, 1], F32, tag="ts")
                nc.vector.scalar_tensor_tensor(
                    out=t[:], in0=pu[:], scalar=thr[:], in1=pu[:],
                    op0=ALU.is_gt, op1=ALU.mult, accum_out=ts[:])
                rinv = stat.tile([128, 1], F32, tag="rinv")
                nc.vector.reciprocal(rinv[:], ts[:])
                # re_attn = t * (1/ts); DMA on the ACT HWDGE queue
                re = work.tile([128, C], F32, tag="re")
                nc.scalar.activation(re[:], t[:], ACTF.Copy, bias=0.0, scale=rinv[:])
                nc.sync.dma_start(out=re_out[b], in_=re[:])

                # ---- PE: bmm1(b+1) fills the chain gap ----
                if b + 1 < BPC:
                    a_cur = bmm1(b + 1)

                # ---- t^T (bf16 transposes into a ps_tp slot) ----
                tT = qp.tile([128, 8, Q], BF16, tag="tT")
                tpf = ps_tp.tile([128, 16, 128], BF16, tag="tp", name="tpf")
                for jc in range(8):
                    nc.tensor.transpose(
                        tpf[:, jc, :],
                        t[:, jc * 128:(jc + 1) * 128], identb[:])
                nc.vector.tensor_copy(
                    tT[:].rearrange("p a b -> p (a b)"),
                    tpf[:, 0:8, :].rearrange("p a b -> p (a b)"))

                # ---- bmm2: wc = (t @ ctx) * rinv ----
                ctx = ctx_t[b]
                for jc in range(8):
                    st, sp = jc == 0, jc == 7
                    nc.tensor.matmul(w0[:], tT[:, jc, :], ctx[:, jc, 0:512],
                                     start=st, stop=sp)
                    nc.tensor.matmul(w2[:], tT[:, jc, :], ctx[:, jc, 512:1024],
                                     start=st, stop=sp)
                wc = work.tile([128, D], F32, tag="wc")
                nc.scalar.activation(wc[:, 0:512], w0[:], ACTF.Copy,
                                     bias=0.0, scale=rinv[:])
                nc.scalar.activation(wc[:, 512:1024], w2[:], ACTF.Copy,
                                     bias=0.0, scale=rinv[:])
                nc.sync.dma_start(out=wc_out[b], in_=wc[:])
                ctx_t[b] = None
                ctxT_t[b] = None
                qT_t[b] = None

    nc.compile()
    return nc


def kernel(query: np.ndarray, context: np.ndarray):
    query = np.ascontiguousarray(query, dtype=np.float32)
    context = np.ascontiguousarray(context, dtype=np.float32)
    assert query.shape == (NB, Q, D) and context.shape == (NB, C, D)

    if "nc" not in _CACHE:
        _CACHE["nc"] = _build()
    nc = _CACHE["nc"]

    bf16 = ml_dtypes.bfloat16
    # qT host prep: (B, Q, D) -> [b, p, jd, q] where d = jd*128 + p
    qT = np.ascontiguousarray(
        query.transpose(0, 2, 1).reshape(NB, 8, 128, Q).transpose(0, 2, 1, 3)
    ).astype(bf16)
    # context: (B, C, D) -> [b, p, jc, d] with c = jc*128 + p
    ctx_bf = np.ascontiguousarray(
        context.reshape(NB, 8, 128, D).transpose(0, 2, 1, 3)
    ).astype(bf16)

    in_maps = []
    for k in range(NCORES):
        sl = slice(k * BPC, (k + 1) * BPC)
        in_maps.append({"query": qT[sl], "context": ctx_bf[sl]})

    trace = os.environ.get("KERNEL_TRACE", "0") == "1"
    res = run_bass_kernel_spmd(nc, in_maps, core_ids=list(range(NCORES)),
                               trace=trace)
    _CACHE["last_res"] = res

    re_attn = np.concatenate([r["re_attn"] for r in res.results], axis=0)
    wcontext = np.concatenate([r["wcontext"] for r in res.results], axis=0)
    return query, wcontext, re_attn


# revision 19
# speedup vs baseline: 1.0096x; 1.0096x over previous
"""Trainium2 Bass kernel for nn_AttnBFAN (batched attention w/ focal re-norm).

Data-parallel over the batch dim: 128 batches sharded 16-per-core across 8
NeuronCores. Per batch (Q=128, C=1024, D=1024):
    attn = leaky_relu(context @ query^T, 0.1)          (C, Q)
    attn = attn / (||attn||_2 over q)                  l2norm per (b, c)
    p    = softmax(20 * attn^T, axis=c)                (Q, C)
    t    = (p > mean_c p) * p ; re_attn = t / sum_c t
    wcontext = re_attn @ context                       (Q, D)
returns (query, wcontext, re_attn).

v4: bf16 matmul path + aggressive software pipelining.
 - Host pre-casts context/query to bf16 (halves HBM traffic; rel err vs
   fp32 reference ~6e-3) and pre-transposes query to [d, q] chunks.
 - All PE transposes and bmms run in bf16 at 1.0 cycle/row. The l2-norm
   / softmax / focal chain stays f32 (per-c-column norm errors don't
   cancel in the softmax).
 - bmm2 multiplies the unnormalized focal weights t (bf16) and folds
   the 1/sum_c(t) renorm into the PSUM eviction (per-partition scale).
 - PE stream per batch: [ctx^T chunks for b+1] | ones(b) | bmm1(b+1) |
   t^T(b) | bmm2(b) — bmm1 of the next batch fills the softmax-chain
   gap. The l2/softmax PSUM (S) lives in the bmm2 banks so bmm1(b+1)
   can take a0/a1 right after the Prelu eviction.
 - Loads/stores spread over the three dynamic DMA queues (gpsimd SWDGE,
   SP HWDGE, ACT HWDGE).
"""

import os
import numpy as np
import ml_dtypes

import concourse.bacc as bacc
import concourse.mybir as mybir
import concourse.tile as tile
from concourse.bass_utils import run_bass_kernel_spmd
from concourse.masks import make_identity
from concourse.hw_specs import get_activation_tables

F32 = mybir.dt.float32
F32R = mybir.dt.float32r
BF16 = mybir.dt.bfloat16
AX = mybir.AxisListType
ALU = mybir.AluOpType
ACTF = mybir.ActivationFunctionType

NCORES = 8
NB = 128          # total batches
BPC = NB // NCORES  # batches per core
Q = 128
C = 1024
D = 1024
SMOOTH = 20.0

_CACHE = {}
STAGES = []  # (label, first_instruction_id) build-time markers for tracing


def _build():
    nc = bacc.Bacc("TRN2", target_bir_lowering=False, debug=False,
                   num_devices=NCORES, name="attn_bfan")

    def mark(label):
        STAGES.append((label, int(nc.get_next_instruction_name().split("-")[1])))
    # query pre-transposed+tiled on host: [b, p(=d%128), jd, q] bf16
    q_in = nc.dram_tensor("query", [BPC, 128, 8, Q], BF16, kind="ExternalInput")
    # context pre-tiled on host: [b, p(=c%128), jc, d] bf16 so each
    # partition's DMA line is 16 KB contiguous (descriptor-overhead bound
    # otherwise: 2 KB bf16 rows halve effective DMA rate)
    c_in = nc.dram_tensor("context", [BPC, 128, 8, D], BF16, kind="ExternalInput")
    re_out = nc.dram_tensor("re_attn", [BPC, Q, C], F32, kind="ExternalOutput")
    wc_out = nc.dram_tensor("wcontext", [BPC, Q, D], F32, kind="ExternalOutput")

    with tile.TileContext(nc) as tc:
        with (
            tc.tile_pool(name="singles", bufs=1) as singles,
            tc.tile_pool(name="ctxp", bufs=3) as ctxp,
            tc.tile_pool(name="ctxtp", bufs=2) as ctxtp,
            tc.tile_pool(name="qp", bufs=2) as qp,
            tc.tile_pool(name="work", bufs=2) as work,
            tc.tile_pool(name="w1", bufs=1) as w1,
            tc.tile_pool(name="tpool", bufs=2) as tpool,
            tc.tile_pool(name="stat", bufs=2) as stat,
            tc.tile_pool(name="ps_a", bufs=1, space="PSUM") as ps_a,
            tc.tile_pool(name="ps_w", bufs=1, space="PSUM") as ps_w,
            tc.tile_pool(name="ps_tp", bufs=2, space="PSUM") as ps_tp,
        ):
            tab_names = list(get_activation_tables("gen3").keys())
            nc.scalar.add_instruction(mybir.InstLoadActFuncSet(
                name=nc.get_next_instruction_name(),
                act_func_set_id=tab_names.index("natural_log_exp_and_others"),
                ins=[], outs=[]))
            ident = singles.tile([128, 128], F32, tag="ident")
            make_identity(nc, ident[:])
            identb = singles.tile([128, 128], BF16, tag="identb")
            nc.vector.tensor_copy(identb[:], ident[:])
            ones_f = singles.tile([128, 128], F32, tag="ones_f")
            nc.vector.memset(ones_f[:], 1.0)
            ones_r = singles.tile([128, 128], F32R, tag="ones_r")
            nc.vector.tensor_copy(ones_r[:], ones_f[:])
            ln20 = singles.tile([128, 1], F32, tag="ln20")
            nc.vector.memset(ln20[:], float(np.log(SMOOTH)))
            invC = singles.tile([128, 1], F32, tag="invC")
            nc.vector.memset(invC[:], 1.0 / C)

            ctx_t = [None] * (BPC + 1)   # plain ctx bf16 [128, 8jc, 1024d]
            ctxT_t = [None] * (BPC + 1)  # ctx^T bf16 [128, 8jd, 1024c]
            qT_t = [None] * (BPC + 1)    # q^T bf16 [128, 8jd, 128q]

            def load_batch(b):
                ctx = ctxp.tile([128, 8, D], BF16, tag="ctx", name="ctx")
                nc.gpsimd.dma_start(out=ctx[:], in_=c_in[b])
                ctx_t[b] = ctx
                qT = qp.tile([128, 8, Q], BF16, tag="qT", name="qT")
                nc.gpsimd.dma_start(out=qT[:], in_=q_in[b])
                qT_t[b] = qT

            def transpose_pair(b, jd, copy_eng):
                # PE-transpose ctx chunks jd, jd+1 into one 2-bank PSUM tile,
                # evict with a single 2048-elem bf16 copy on copy_eng.
                ctx = ctx_t[b]
                if ctxT_t[b] is None:
                    ctxT_t[b] = ctxtp.tile([128, 8, C], BF16, tag="ctxT",
                                           name="ctxT")
                ctxT = ctxT_t[b]
                tp = ps_tp.tile([128, 16, 128], BF16, tag="tp", name="tp")
                for k in range(2):
                    for jc in range(8):
                        nc.tensor.transpose(
                            tp[:, k * 8 + jc, :],
                            ctx[:, jc, (jd + k) * 128:(jd + k + 1) * 128],
                            identb[:])
                src = tp[:].rearrange("p a b -> p (a b)")
                dst = ctxT[:, jd:jd + 2, :].rearrange("p a b -> p (a b)")
                if copy_eng == "act":
                    nc.scalar.copy(dst, src)
                else:
                    nc.vector.tensor_copy(dst, src)

            def bmm1(b):
                # attn^T (q, c) accumulated over 8 d-chunks -> a0/a1
                a0 = ps_a.tile([128, 512], F32, tag="a0", name="a0")
                a1 = ps_a.tile([128, 512], F32, tag="a1", name="a1")
                qT = qT_t[b]
                ctxT = ctxT_t[b]
                for jd in range(8):
                    st, sp = jd == 0, jd == 7
                    nc.tensor.matmul(a0[:], qT[:, jd, :], ctxT[:, jd, 0:512],
                                     start=st, stop=sp)
                    nc.tensor.matmul(a1[:], qT[:, jd, :], ctxT[:, jd, 512:1024],
                                     start=st, stop=sp)
                return a0, a1

            # ---- prologue: batch 0 fully staged, batch 1 loading ----
            load_batch(0)
            load_batch(1)
            for jd in (0, 2, 4, 6):
                transpose_pair(0, jd, "vec" if jd != 0 else "act")
            a_cur = bmm1(0)

            for b in range(BPC):
                a0, a1 = a_cur
                mark(f'iter{b}_start')
                if b + 2 < BPC:
                    load_batch(b + 2)

                # ---- leaky relu eviction (frees a0/a1 for bmm1(b+1)) ----
                mark(f'i{b}_prelu')
                attn = work.tile([128, C], F32, tag="attn")
                nc.scalar.activation(attn[:, 0:512], a0[:], ACTF.Prelu,
                                     bias=0.0, scale=1.0, alpha=0.1)
                nc.scalar.activation(attn[:, 512:1024], a1[:], ACTF.Prelu,
                                     bias=0.0, scale=1.0, alpha=0.1)
                # squares on DVE (f32r for the ones-matmul)
                mark(f'i{b}_sq')
                sq = w1.tile([128, C], F32R, tag="w1a")
                nc.vector.tensor_mul(sq[:, 0:512], attn[:, 0:512], attn[:, 0:512])
                nc.vector.tensor_mul(sq[:, 512:1024], attn[:, 512:1024],
                                     attn[:, 512:1024])

                # next batch ctx^T chunks 0-3 (ACT copies the first pair in
                # its pre-Ln gap, DVE the second)
                if b + 1 < BPC:
                    mark(f'i{b}_tp02')
                    transpose_pair(b + 1, 0, "act")
                    transpose_pair(b + 1, 2, "vec")

                # ---- l2 norm: ones-matmul into the bmm2 banks ----
                mark(f'i{b}_ones')
                w0 = ps_w.tile([128, 512], F32, tag="w0", name="w0")
                w2 = ps_w.tile([128, 512], F32, tag="w2", name="w2")
                nc.tensor.matmul(w0[:], ones_r[:], sq[:, 0:512], start=True, stop=True)
                nc.tensor.matmul(w2[:], ones_r[:], sq[:, 512:1024], start=True, stop=True)

                if b + 1 < BPC:
                    mark(f'i{b}_tp46')
                    transpose_pair(b + 1, 4, "vec")
                    transpose_pair(b + 1, 6, "vec")

                # 20/sqrt(S) = exp(-0.5*ln(S) + ln 20), half-split so ACT/DVE
                # ping-pong: Ln0 -> rn0 -> u0(DVE) -> pu0 while h1 follows
                mark(f'i{b}_ln')
                lnS = w1.tile([128, C], F32, tag="w1b")
                nc.scalar.activation(lnS[:, 0:512], w0[:], ACTF.Ln)
                nc.scalar.activation(lnS[:, 512:1024], w2[:], ACTF.Ln)
                rn20 = w1.tile([128, C], F32, tag="w1c")
                u = w1.tile([128, C], F32, tag="w1a")
                pu = work.tile([128, C], F32, tag="pu")
                rs0 = stat.tile([128, 1], F32, tag="rs0")
                rs1 = stat.tile([128, 1], F32, tag="rs1")
                h0, h1 = slice(0, 512), slice(512, 1024)
                nc.scalar.activation(rn20[:, h0], lnS[:, h0], ACTF.Exp,
                                     bias=ln20[:], scale=-0.5)
                nc.scalar.activation(rn20[:, h1], lnS[:, h1], ACTF.Exp,
                                     bias=ln20[:], scale=-0.5)
                nc.vector.tensor_mul(u[:, h0], attn[:, h0], rn20[:, h0])
                nc.vector.tensor_mul(u[:, h1], attn[:, h1], rn20[:, h1])
                # pu = exp(20u), rs = sum_c pu per half
                nc.scalar.activation(pu[:, h0], u[:, h0], ACTF.Exp,
                                     bias=0.0, scale=1.0, accum_out=rs0[:])
                nc.scalar.activation(pu[:, h1], u[:, h1], ACTF.Exp,
                                     bias=0.0, scale=1.0, accum_out=rs1[:])
                # thr = (rs0 + rs1) / C in one DVE op
                thr = stat.tile([128, 1], F32, tag="thr")
                nc.vector.scalar_tensor_tensor(
                    out=thr[:], in0=rs0[:], scalar=rs1[:], in1=invC[:],
                    op0=ALU.add, op1=ALU.mult)

                # ---- focal: t = (pu > thr) * pu (bf16), ts = sum_c t ----
                mark(f'i{b}_focal')
                t = tpool.tile([128, C], BF16, tag="t")
                ts = stat.tile([128, 1], F32, tag="ts")
                nc.vector.scalar_tensor_tensor(
                    out=t[:], in0=pu[:], scalar=thr[:], in1=pu[:],
                    op0=ALU.is_gt, op1=ALU.mult, accum_out=ts[:])
                rinv = stat.tile([128, 1], F32, tag="rinv")
                nc.vector.reciprocal(rinv[:], ts[:])
                # re_attn = t * (1/ts); DMA on the ACT HWDGE queue
                re = work.tile([128, C], F32, tag="re")
                nc.scalar.activation(re[:], t[:], ACTF.Copy, bias=0.0, scale=rinv[:])
                nc.sync.dma_start(out=re_out[b], in_=re[:])

                # ---- PE: bmm1(b+1) fills the chain gap ----
                if b + 1 < BPC:
                    mark(f'i{b}_bmm1n')
                    a_cur = bmm1(b + 1)

                # ---- t^T (bf16 transposes into a ps_tp slot) ----
                mark(f'i{b}_tT')
                tT = qp.tile([128, 8, Q], BF16, tag="tT")
                tpf = ps_tp.tile([128, 16, 128], BF16, tag="tp", name="tpf")
                for jc in range(8):
                    nc.tensor.transpose(
                        tpf[:, jc, :],
                        t[:, jc * 128:(jc + 1) * 128], identb[:])
                nc.vector.tensor_copy(
                    tT[:].rearrange("p a b -> p (a b)"),
                    tpf[:, 0:8, :].rearrange("p a b -> p (a b)"))

                # ---- bmm2: wc = (t @ ctx) * rinv ----
                mark(f'i{b}_bmm2')
                ctx = ctx_t[b]
                for jc in range(8):
                    st, sp = jc == 0, jc == 7
                    nc.tensor.matmul(w0[:], tT[:, jc, :], ctx[:, jc, 0:512],
                                     start=st, stop=sp)
                    nc.tensor.matmul(w2[:], tT[:, jc, :], ctx[:, jc, 512:1024],
                                     start=st, stop=sp)
                mark(f'i{b}_wc')
                wc = work.tile([128, D], F32, tag="wc")
                nc.scalar.activation(wc[:, 0:512], w0[:], ACTF.Copy,
                                     bias=0.0, scale=rinv[:])
                nc.scalar.activation(wc[:, 512:1024], w2[:], ACTF.Copy,
                                     bias=0.0, scale=rinv[:])
                nc.sync.dma_start(out=wc_out[b], in_=wc[:])
                ctx_t[b] = None
                ctxT_t[b] = None
                qT_t[b] = None

    nc.compile()
    return nc


def kernel(query: np.ndarray, context: np.ndarray):
    query = np.ascontiguousarray(query, dtype=np.float32)
    context = np.ascontiguousarray(context, dtype=np.float32)
    assert query.shape == (NB, Q, D) and context.shape == (NB, C, D)

    if "nc" not in _CACHE:
        _CACHE["nc"] = _build()
    nc = _CACHE["nc"]

    bf16 = ml_dtypes.bfloat16
    # qT host prep: (B, Q, D) -> [b, p, jd, q] where d = jd*128 + p
    qT = np.ascontiguousarray(
        query.transpose(0, 2, 1).reshape(NB, 8, 128, Q).transpose(0, 2, 1, 3)
    ).astype(bf16)
    # context: (B, C, D) -> [b, p, jc, d] with c = jc*128 + p
    ctx_bf = np.ascontiguousarray(
        context.reshape(NB, 8, 128, D).transpose(0, 2, 1, 3)
    ).astype(bf16)

    in_maps = []
    for k in range(NCORES):
        sl = slice(k * BPC, (k + 1) * BPC)
        in_maps.append({"query": qT[sl], "context": ctx_bf[sl]})

    trace = os.environ.get("KERNEL_TRACE", "0") == "1"
    res = run_bass_kernel_spmd(nc, in_maps, core_ids=list(range(NCORES)),
                               trace=trace)
    _CACHE["last_res"] = res

    re_attn = np.concatenate([r["re_attn"] for r in res.results], axis=0)
    wcontext = np.concatenate([r["wcontext"] for r in res.results], axis=0)
    return query, wcontext, re_attn


# revision 22
# speedup vs baseline: 1.2227x; 1.2111x over previous
"""Trainium2 Bass kernel for nn_AttnBFAN (batched attention w/ focal re-norm).

Data-parallel over the batch dim: 128 batches sharded 16-per-core across 8
NeuronCores. Per batch (Q=128, C=1024, D=1024):
    attn = leaky_relu(context @ query^T, 0.1)          (C, Q)
    attn = attn / (||attn||_2 over q)                  l2norm per (b, c)
    p    = softmax(20 * attn^T, axis=c)                (Q, C)
    t    = (p > mean_c p) * p ; re_attn = t / sum_c t
    wcontext = re_attn @ context                       (Q, D)
returns (query, wcontext, re_attn).

v8: bf16 matmul path + two-level software pipelining.
 - Host pre-casts context/query to bf16 (halves HBM traffic; rel err vs
   fp32 reference ~6e-3), pre-transposes query to [d, q] chunks, and
   pre-tiles context so each partition's DMA line is 16 KB contiguous.
 - All PE transposes and bmms run in bf16 at 1.0 cycle/row. The l2-norm
   / softmax / focal chain stays f32 (per-c-column norm errors don't
   cancel in the softmax).
 - bmm2 multiplies the unnormalized focal weights t (bf16) and folds
   the 1/sum_c(t) renorm into the PSUM eviction (per-partition scale).
 - PE stream per batch b: T4-7(b+1) | ones(b) | bmm1(b+1) | T0-3(b+2) |
   t^T(b) | bmm2(b). The next batch's bmm1 and the batch-after-next's
   first transposes fill the softmax-chain latency so the PE never
   idles (and stays at the 2.4 GHz p-state). PSUM: 3-deep single-bank
   ring for ctx^T staging, 1 bank for t^T, 2+2 banks for the bmms (the
   l2 sums share the bmm2 banks so bmm1(b+1) can take a0/a1 right
   after the Prelu eviction).
"""

import os
import numpy as np
import ml_dtypes

import concourse.bacc as bacc
import concourse.mybir as mybir
import concourse.tile as tile
from concourse.bass_utils import run_bass_kernel_spmd
from concourse.masks import make_identity
from concourse.hw_specs import get_activation_tables

F32 = mybir.dt.float32
F32R = mybir.dt.float32r
BF16 = mybir.dt.bfloat16
AX = mybir.AxisListType
ALU = mybir.AluOpType
ACTF = mybir.ActivationFunctionType

NCORES = 8
NB = 128          # total batches
BPC = NB // NCORES  # batches per core
Q = 128
C = 1024
D = 1024
SMOOTH = 20.0

_CACHE = {}
STAGES = []  # (label, first_instruction_id) build-time markers for tracing


def _build():
    nc = bacc.Bacc("TRN2", target_bir_lowering=False, debug=False,
                   num_devices=NCORES, name="attn_bfan")

    def mark(label):
        STAGES.append((label, int(nc.get_next_instruction_name().split("-")[1])))

    # query pre-transposed+tiled on host: [b, p(=d%128), jd, q] bf16
    q_in = nc.dram_tensor("query", [BPC, 128, 8, Q], BF16, kind="ExternalInput")
    # context pre-tiled on host: [b, p(=c%128), jc, d] bf16
    c_in = nc.dram_tensor("context", [BPC, 128, 8, D], BF16, kind="ExternalInput")
    re_out = nc.dram_tensor("re_attn", [BPC, Q, C], F32, kind="ExternalOutput")
    wc_out = nc.dram_tensor("wcontext", [BPC, Q, D], F32, kind="ExternalOutput")

    with tile.TileContext(nc) as tc:
        with (
            tc.tile_pool(name="singles", bufs=1) as singles,
            tc.tile_pool(name="ctxp", bufs=4) as ctxp,
            tc.tile_pool(name="ctxtp", bufs=2) as ctxtp,
            tc.tile_pool(name="qTp", bufs=3) as qTp,
            tc.tile_pool(name="tTp", bufs=2) as tTp,
            tc.tile_pool(name="work", bufs=2) as work,
            tc.tile_pool(name="w1", bufs=1) as w1,
            tc.tile_pool(name="tpool", bufs=2) as tpool,
            tc.tile_pool(name="stat", bufs=2) as stat,
            tc.tile_pool(name="ps_a", bufs=1, space="PSUM") as ps_a,
            tc.tile_pool(name="ps_w", bufs=1, space="PSUM") as ps_w,
            tc.tile_pool(name="ps_f", bufs=1, space="PSUM") as ps_f,
            tc.tile_pool(name="ps_tp", bufs=3, space="PSUM") as ps_tp,
        ):
            tab_names = list(get_activation_tables("gen3").keys())
            nc.scalar.add_instruction(mybir.InstLoadActFuncSet(
                name=nc.get_next_instruction_name(),
                act_func_set_id=tab_names.index("natural_log_exp_and_others"),
                ins=[], outs=[]))
            ident = singles.tile([128, 128], F32, tag="ident")
            make_identity(nc, ident[:])
            identb = singles.tile([128, 128], BF16, tag="identb")
            nc.vector.tensor_copy(identb[:], ident[:])
            ones_f = singles.tile([128, 128], F32, tag="ones_f")
            nc.vector.memset(ones_f[:], 1.0)
            ones_r = singles.tile([128, 128], F32R, tag="ones_r")
            nc.vector.tensor_copy(ones_r[:], ones_f[:])
            ln20 = singles.tile([128, 1], F32, tag="ln20")
            nc.vector.memset(ln20[:], float(np.log(SMOOTH)))
            invC = singles.tile([128, 1], F32, tag="invC")
            nc.vector.memset(invC[:], 1.0 / C)

            ctx_t = [None] * (BPC + 3)   # plain ctx bf16 [128, 8jc, 1024d]
            ctxT_t = [None] * (BPC + 3)  # ctx^T bf16 [128, 8jd, 1024c]
            qT_t = [None] * (BPC + 3)    # q^T bf16 [128, 8jd, 128q]

            def load_batch(b):
                ctx = ctxp.tile([128, 8, D], BF16, tag="ctx", name="ctx")
                nc.gpsimd.dma_start(out=ctx[:], in_=c_in[b])
                ctx_t[b] = ctx
                qT = qTp.tile([128, 8, Q], BF16, tag="qT", name="qT")
                nc.gpsimd.dma_start(out=qT[:], in_=q_in[b])
                qT_t[b] = qT

            def transpose_jd_pe(b, jd):
                # PE-transpose ctx d-chunk jd into a 1-bank PSUM tile
                ctx = ctx_t[b]
                if ctxT_t[b] is None:
                    ctxT_t[b] = ctxtp.tile([128, 8, C], BF16, tag="ctxT",
                                           name="ctxT")
                tp = ps_tp.tile([128, 8, 128], BF16, tag="tp", name="tp")
                for jc in range(8):
                    nc.tensor.transpose(
                        tp[:, jc, :],
                        ctx[:, jc, jd * 128:(jd + 1) * 128], identb[:])
                return tp

            def copy_jd(b, jd, tp, copy_eng):
                # evict one transposed d-chunk: 1024-elem bf16 PSUM->SBUF copy
                src = tp[:].rearrange("p a b -> p (a b)")
                if copy_eng == "act":
                    nc.scalar.copy(ctxT_t[b][:, jd, :], src)
                else:
                    nc.vector.tensor_copy(ctxT_t[b][:, jd, :], src)

            def transpose_jd(b, jd, copy_eng):
                copy_jd(b, jd, transpose_jd_pe(b, jd), copy_eng)

            def bmm1(b):
                # attn^T (q, c) accumulated over 8 d-chunks -> a0/a1
                a0 = ps_a.tile([128, 512], F32, tag="a0", name="a0")
                a1 = ps_a.tile([128, 512], F32, tag="a1", name="a1")
                qT = qT_t[b]
                ctxT = ctxT_t[b]
                for jd in range(8):
                    st, sp = jd == 0, jd == 7
                    nc.tensor.matmul(a0[:], qT[:, jd, :], ctxT[:, jd, 0:512],
                                     start=st, stop=sp)
                    nc.tensor.matmul(a1[:], qT[:, jd, :], ctxT[:, jd, 512:1024],
                                     start=st, stop=sp)
                return a0, a1

            # ---- prologue: batch 0 fully staged, 1 half-transposed ----
            load_batch(0)
            load_batch(1)
            load_batch(2)
            for jd in range(8):
                transpose_jd(0, jd, "vec" if jd % 4 != 3 else "act")
            for jd in range(4):
                transpose_jd(1, jd, "vec" if jd % 4 != 3 else "act")
            a_cur = bmm1(0)

            for b in range(BPC):
                a0, a1 = a_cur
                mark(f'iter{b}')
                if b + 3 < BPC:
                    load_batch(b + 3)

                # ---- leaky relu eviction (frees a0/a1 for bmm1(b+1)) ----
                mark(f'i{b}_prelu')
                attn = work.tile([128, C], F32, tag="attn")
                nc.scalar.activation(attn[:, 0:512], a0[:], ACTF.Prelu,
                                     bias=0.0, scale=1.0, alpha=0.1)
                nc.scalar.activation(attn[:, 512:1024], a1[:], ACTF.Prelu,
                                     bias=0.0, scale=1.0, alpha=0.1)
                sq = w1.tile([128, C], F32R, tag="w1a")
                nc.vector.tensor_mul(sq[:, 0:512], attn[:, 0:512], attn[:, 0:512])
                nc.vector.tensor_mul(sq[:, 512:1024], attn[:, 512:1024],
                                     attn[:, 512:1024])

                # finish next batch's ctx^T (jd 4-7); cpy4/6 on ACT fill its
                # pre-Ln slack, cpy5/7 on DVE after the squares
                if b + 1 < BPC:
                    mark(f'i{b}_tp47')
                    transpose_jd(b + 1, 4, "act")
                    transpose_jd(b + 1, 5, "vec")
                    transpose_jd(b + 1, 6, "act")
                    transpose_jd(b + 1, 7, "vec")

                # ---- l2 norm: ones-matmul into the bmm2 banks ----
                mark(f'i{b}_ones')
                w0 = ps_w.tile([128, 512], F32, tag="w0", name="w0")
                w2 = ps_w.tile([128, 512], F32, tag="w2", name="w2")
                nc.tensor.matmul(w0[:], ones_r[:], sq[:, 0:512], start=True, stop=True)
                nc.tensor.matmul(w2[:], ones_r[:], sq[:, 512:1024], start=True, stop=True)

                # 20/sqrt(S) = exp(-0.5*ln(S) + ln 20), half-split ACT/DVE
                mark(f'i{b}_ln')
                lnS = w1.tile([128, C], F32, tag="w1b")
                nc.scalar.activation(lnS[:, 0:512], w0[:], ACTF.Ln)
                nc.scalar.activation(lnS[:, 512:1024], w2[:], ACTF.Ln)
                rn20 = w1.tile([128, C], F32, tag="w1c")
                u = w1.tile([128, C], F32, tag="w1a")
                pu = work.tile([128, C], F32, tag="pu")
                rs0 = stat.tile([128, 1], F32, tag="rs0")
                rs1 = stat.tile([128, 1], F32, tag="rs1")
                h0, h1 = slice(0, 512), slice(512, 1024)
                nc.scalar.activation(rn20[:, h0], lnS[:, h0], ACTF.Exp,
                                     bias=ln20[:], scale=-0.5)
                nc.scalar.activation(rn20[:, h1], lnS[:, h1], ACTF.Exp,
                                     bias=ln20[:], scale=-0.5)
                nc.vector.tensor_mul(u[:, h0], attn[:, h0], rn20[:, h0])
                nc.vector.tensor_mul(u[:, h1], attn[:, h1], rn20[:, h1])
                nc.scalar.activation(pu[:, h0], u[:, h0], ACTF.Exp,
                                     bias=0.0, scale=1.0, accum_out=rs0[:])
                nc.scalar.activation(pu[:, h1], u[:, h1], ACTF.Exp,
                                     bias=0.0, scale=1.0, accum_out=rs1[:])
                # thr = (rs0 + rs1) / C in one DVE op
                thr = stat.tile([128, 1], F32, tag="thr")
                nc.vector.scalar_tensor_tensor(
                    out=thr[:], in0=rs0[:], scalar=rs1[:], in1=invC[:],
                    op0=ALU.add, op1=ALU.mult)

                # ---- focal: t = (pu > thr) * pu (bf16), ts = sum_c t ----
                mark(f'i{b}_focal')
                t = tpool.tile([128, C], BF16, tag="t")
                ts = stat.tile([128, 1], F32, tag="ts")
                nc.vector.scalar_tensor_tensor(
                    out=t[:], in0=pu[:], scalar=thr[:], in1=pu[:],
                    op0=ALU.is_gt, op1=ALU.mult, accum_out=ts[:])
                rinv = stat.tile([128, 1], F32, tag="rinv")
                nc.vector.reciprocal(rinv[:], ts[:])
                # re_attn = t * (1/ts)
                re = work.tile([128, C], F32, tag="re")
                nc.scalar.activation(re[:], t[:], ACTF.Copy, bias=0.0, scale=rinv[:])
                nc.sync.dma_start(out=re_out[b], in_=re[:])

                # ---- PE: bmm1(b+1) + first transposes of b+2 fill the
                #      chain-tail gap ----
                if b + 1 < BPC:
                    mark(f'i{b}_bmm1n')
                    a_cur = bmm1(b + 1)
                # first transposes of b+2 fill the chain tail on the PE; the
                # ACT copies run in its post-pu slack, the DVE copies are
                # emitted after tTcpy so they never delay bmm2
                tp_late = []
                if b + 2 < BPC:
                    mark(f'i{b}_tp03')
                    transpose_jd(b + 2, 0, "act")
                    tp_late.append((1, transpose_jd_pe(b + 2, 1)))
                    transpose_jd(b + 2, 2, "act")
                    tp_late.append((3, transpose_jd_pe(b + 2, 3)))

                # ---- t^T (bf16 transposes into the dedicated bank) ----
                mark(f'i{b}_tT')
                tT = tTp.tile([128, 8, Q], BF16, tag="tT")
                tpf = ps_f.tile([128, 8, 128], BF16, tag="tpf")
                for jc in range(8):
                    nc.tensor.transpose(
                        tpf[:, jc, :],
                        t[:, jc * 128:(jc + 1) * 128], identb[:])
                nc.vector.tensor_copy(
                    tT[:].rearrange("p a b -> p (a b)"),
                    tpf[:].rearrange("p a b -> p (a b)"))
                for jd, tp in tp_late:
                    copy_jd(b + 2, jd, tp, "vec")

                # ---- bmm2: wc = (t @ ctx) * rinv ----
                mark(f'i{b}_bmm2')
                ctx = ctx_t[b]
                for jc in range(8):
                    st, sp = jc == 0, jc == 7
                    nc.tensor.matmul(w0[:], tT[:, jc, :], ctx[:, jc, 0:512],
                                     start=st, stop=sp)
                    nc.tensor.matmul(w2[:], tT[:, jc, :], ctx[:, jc, 512:1024],
                                     start=st, stop=sp)
                mark(f'i{b}_wc')
                wc = work.tile([128, D], F32, tag="wc")
                nc.scalar.activation(wc[:, 0:512], w0[:], ACTF.Copy,
                                     bias=0.0, scale=rinv[:])
                nc.scalar.activation(wc[:, 512:1024], w2[:], ACTF.Copy,
                                     bias=0.0, scale=rinv[:])
                nc.sync.dma_start(out=wc_out[b], in_=wc[:])
                ctx_t[b] = None
                ctxT_t[b] = None
                qT_t[b] = None

    nc.compile()
    return nc


def kernel(query: np.ndarray, context: np.ndarray):
    query = np.ascontiguousarray(query, dtype=np.float32)
    context = np.ascontiguousarray(context, dtype=np.float32)
    assert query.shape == (NB, Q, D) and context.shape == (NB, C, D)

    if "nc" not in _CACHE:
        _CACHE["nc"] = _build()
    nc = _CACHE["nc"]

    bf16 = ml_dtypes.bfloat16
    # qT host prep: (B, Q, D) -> [b, p, jd, q] where d = jd*128 + p
    qT = np.ascontiguousarray(
        query.transpose(0, 2, 1).reshape(NB, 8, 128, Q).transpose(0, 2, 1, 3)
    ).astype(bf16)
    # context: (B, C, D) -> [b, p, jc, d] with c = jc*128 + p
    ctx_bf = np.ascontiguousarray(
        context.reshape(NB, 8, 128, D).transpose(0, 2, 1, 3)
    ).astype(bf16)

    in_maps = []
    for k in range(NCORES):
        sl = slice(k * BPC, (k + 1) * BPC)
        in_maps.append({"query": qT[sl], "context": ctx_bf[sl]})

    trace = os.environ.get("KERNEL_TRACE", "0") == "1"
    res = run_bass_kernel_spmd(nc, in_maps, core_ids=list(range(NCORES)),
                               trace=trace)
    _CACHE["last_res"] = res

    re_attn = np.concatenate([r["re_attn"] for r in res.results], axis=0)
    wcontext = np.concatenate([r["wcontext"] for r in res.results], axis=0)
    return query, wcontext, re_attn
